# revision 1
# baseline (speedup 1.0000x reference)
"""Trainium2 Bass kernel for CorrectedPartialCharges.

out[i] = pc[i] + (total_charge[g] - seg_sum[g]) / n_atoms[g],  g = i // 256

Sharding: graphs are data-parallel across the 8 cores (4096 graphs /
1,048,576 atoms per core); segment sums and the gather-broadcast stay
device-local. On each core, partition p owns 32 contiguous graphs, so a
per-graph segment sum is a free-axis reduce over [128, K, 256] tiles and
the correction is a zero-stride broadcast add.
"""

import numpy as np

import concourse.bacc as bacc
import concourse.bass as bass
import concourse.mybir as mybir
import concourse.tile as tile
from concourse.bass_utils import run_bass_kernel_spmd

N_CORES = 8
ATOMS_PER_GRAPH = 256
N_GRAPHS = 32768
N_ATOMS = N_GRAPHS * ATOMS_PER_GRAPH
P = 128

G_PER_CORE = N_GRAPHS // N_CORES          # 4096 graphs per core
A_PER_CORE = G_PER_CORE * ATOMS_PER_GRAPH  # 1,048,576 atoms per core

# Knobs read by test.py when experimenting.
NT = 8  # tiles per core along the free dim

_TRACE = False
_TRACE_KWARGS = {}


def _build(g_per_core: int = G_PER_CORE, nt: int = NT):
    a_per_core = g_per_core * ATOMS_PER_GRAPH
    gp = g_per_core // P          # graphs per partition
    ap_free = a_per_core // P     # atoms per partition
    w = ap_free // nt             # tile width (atoms per partition per tile)
    k = w // ATOMS_PER_GRAPH      # graphs per partition per tile
    assert g_per_core % P == 0 and ap_free % nt == 0 and w % ATOMS_PER_GRAPH == 0

    nc = bacc.Bacc(None, target_bir_lowering=False)

    pc = nc.dram_tensor("pc", [a_per_core], mybir.dt.float32, kind="ExternalInput")
    tch = nc.dram_tensor("tch", [g_per_core], mybir.dt.float32, kind="ExternalInput")
    nat = nc.dram_tensor("nat", [g_per_core], mybir.dt.int32, kind="ExternalInput")
    out = nc.dram_tensor("out", [a_per_core], mybir.dt.float32, kind="ExternalOutput")

    pc_v = pc[:].rearrange("(p n) -> p n", p=P)
    out_v = out[:].rearrange("(p n) -> p n", p=P)
    tch_v = tch[:].rearrange("(p k) -> p k", p=P)
    nat_v = nat[:].rearrange("(p k) -> p k", p=P)

    # Which engine applies the broadcast-add for each tile. The DVE also does
    # all the segment reduces, so most add work is pushed to GpSimd/ACT; ACT
    # additionally issues the output DMAs (second HWDGE queue next to Sync).
    add_engine = ["vector", "scalar", "vector", "scalar", "vector", "scalar", "vector", "scalar"]

    with tile.TileContext(nc) as tc:
        with (
            tc.tile_pool(name="io", bufs=6) as io_pool,
            tc.tile_pool(name="small", bufs=4) as small_pool,
            tc.tile_pool(name="consts", bufs=1) as const_pool,
        ):
            xs = []
            for t in range(min(2, nt)):
                x = io_pool.tile([P, w], mybir.dt.float32, tag="x")
                nc.sync.dma_start(out=x[:], in_=pc_v[:, t * w : (t + 1) * w])
                xs.append(x)

            tc_tile = const_pool.tile([P, gp], mybir.dt.float32, tag="tc")
            nc.sync.dma_start(out=tc_tile[:], in_=tch_v)
            na_i = const_pool.tile([P, gp], mybir.dt.int32, tag="nai")
            nc.sync.dma_start(out=na_i[:], in_=nat_v)
            na_f = const_pool.tile([P, gp], mybir.dt.float32, tag="naf")
            nc.vector.tensor_copy(out=na_f[:], in_=na_i[:])
            rna = const_pool.tile([P, gp], mybir.dt.float32, tag="rna")
            nc.vector.reciprocal(out=rna[:], in_=na_f[:])

            for t in range(nt):
                if t < len(xs):
                    x = xs[t]
                else:
                    x = io_pool.tile([P, w], mybir.dt.float32, tag="x")
                    nc.sync.dma_start(out=x[:], in_=pc_v[:, t * w : (t + 1) * w])
                x3 = x[:].rearrange("p (k a) -> p k a", a=ATOMS_PER_GRAPH)

                seg = small_pool.tile([P, k], mybir.dt.float32, tag="seg")
                nc.vector.reduce_sum(out=seg[:], in_=x3, axis=mybir.AxisListType.X)

                left = small_pool.tile([P, k], mybir.dt.float32, tag="left")
                nc.vector.tensor_sub(
                    out=left[:], in0=tc_tile[:, t * k : (t + 1) * k], in1=seg[:]
                )
                nc.vector.tensor_mul(
                    out=left[:], in0=left[:], in1=rna[:, t * k : (t + 1) * k]
                )

                eng = add_engine[t % len(add_engine)]
                if eng == "vector":
                    lv = left[:]
                    lb = bass.AP(
                        lv.tensor,
                        lv.offset,
                        [list(lv.ap[0]), list(lv.ap[1]), [0, ATOMS_PER_GRAPH]],
                    )
                    nc.vector.tensor_add(out=x3, in0=x3, in1=lb)
                else:
                    for j in range(k):
                        blk = x[:, j * ATOMS_PER_GRAPH : (j + 1) * ATOMS_PER_GRAPH]
                        nc.scalar.add(out=blk, in_=blk, add=left[:, j : j + 1])
                nc.scalar.dma_start(out=out_v[:, t * w : (t + 1) * w], in_=x[:])

    nc.finalize()
    return nc


_NC_CACHE = {}


def _get_nc(g_per_core: int = G_PER_CORE, nt: int = NT):
    key = (g_per_core, nt)
    if key not in _NC_CACHE:
        _NC_CACHE[key] = _build(g_per_core, nt)
    return _NC_CACHE[key]


def _cpu_fallback(pc, total_charge, batch, n_atoms):
    num_segments = n_atoms.shape[0]
    seg = np.bincount(batch, weights=pc.astype(np.float64), minlength=num_segments)
    leftover = (total_charge - seg.astype(np.float32)) / n_atoms.astype(np.float32)
    return (pc + leftover[batch]).astype(np.float32)


def kernel(**inputs) -> np.ndarray:
    pc = np.ascontiguousarray(
        np.asarray(inputs["node_outputs"], dtype=np.float32).reshape(-1)
    )
    total_charge = np.ascontiguousarray(
        np.asarray(inputs["total_charge"], dtype=np.float32).reshape(-1)
    )
    batch = np.asarray(inputs["batch"]).reshape(-1)
    n_atoms = np.ascontiguousarray(np.asarray(inputs["n_atoms"], dtype=np.int32).reshape(-1))

    # The device kernel hardcodes the uniform 256-atoms-per-graph layout the
    # reference generator produces; anything else goes through numpy.
    if (
        pc.shape[0] != N_ATOMS
        or total_charge.shape[0] != N_GRAPHS
        or not np.array_equal(
            batch.astype(np.int64),
            np.arange(N_ATOMS, dtype=np.int64) // ATOMS_PER_GRAPH,
        )
    ):
        return _cpu_fallback(pc, total_charge, batch, n_atoms)

    nc = _get_nc()
    in_maps = []
    for c in range(N_CORES):
        in_maps.append(
            {
                "pc": pc[c * A_PER_CORE : (c + 1) * A_PER_CORE],
                "tch": total_charge[c * G_PER_CORE : (c + 1) * G_PER_CORE],
                "nat": n_atoms[c * G_PER_CORE : (c + 1) * G_PER_CORE],
            }
        )
    res = run_bass_kernel_spmd(
        nc, in_maps, list(range(N_CORES)), trace=_TRACE, **_TRACE_KWARGS
    )
    out = np.concatenate([r["out"] for r in res.results])
    if _TRACE:
        kernel.last_results = res
    return out

